# revision 23
# baseline (speedup 1.0000x reference)
"""Trainium2 Bass kernel for nn_Attention_34033320854122.

Dense transformer attention block: QKV proj -> causal depthwise conv+SiLU ->
per-head RMSNorm -> partial RoPE -> causal attention -> output projection.

Sharding: tensor-parallel over the 16 heads across 8 NeuronCores (2 heads =
256 channels per core). Each core computes q/k/v for its channels (full
contraction over D), runs attention for its 2 heads, and produces a partial
output projection (outT_partial = Wo[:, cols] @ attn_cols^T). The host sums
the 8 partials and transposes.

Fidelity notes:
- The reference negates the rotated RoPE sub-dim of BOTH q and k; the
  negation cancels in q.k and is skipped.
- softmax without max-subtraction: scores are O(1)-bounded.
- rstd = 1/sqrt(mean(x^2)) computed as exp(-0.5*ln(ms)); eps=1e-5 is
  dropped (ms is O(0.1..1), relative impact < 1e-4).
- norm weights are folded into the RoPE trig tables (rot rows) and a
  per-partition scalar (pass rows); rstd is applied post-rope (it is a
  per-position scalar, commuting with the rotation).

Scheduling: activation table-set switches are minimized (silu-set, then
natural-log/exp set for everything else). RoPE's misaligned half-rotation
products run on GpSimd; V is transposed by the DMA XBAR; per-position
reciprocal-norm rows are partition-broadcast by stride-0 DMA.
"""

from contextlib import ExitStack

import ml_dtypes
import numpy as np

import concourse.bacc as bacc
import concourse.tile as tile
import concourse.mybir as mybir
from concourse import bass_utils

# Problem shape (hardcoded per contract)
B, T, D = 1, 2048, 2048
H, HD = 16, 128
RD = 64
KCONV = 4
NCORES = 8
CPC = D // NCORES      # channels per core = 256
MPC = CPC // HD        # head tiles per core = 2
NT = 512               # free-dim tile for matmuls
NQ = T // NT           # 4 q tiles
KD = D // 128          # 16 contraction chunks
PAD = KCONV - 1        # causal conv history
HT = T // 2            # half-span for conv/silu

F32 = mybir.dt.float32
BF16 = mybir.dt.bfloat16

_COMPILED = None
_DEBUG = False
_DEBUG_RESULTS = None


def _build():
    nc = bacc.Bacc("TRN2", target_bir_lowering=False, debug=False,
                   num_devices=NCORES)

    d = {}
    d["xT"] = nc.dram_tensor("xT", (D, T), BF16, kind="ExternalInput").ap()
    d["wqT"] = nc.dram_tensor("wqT", (D, CPC), BF16, kind="ExternalInput").ap()
    d["wkT"] = nc.dram_tensor("wkT", (D, CPC), BF16, kind="ExternalInput").ap()
    d["wvT"] = nc.dram_tensor("wvT", (D, CPC), BF16, kind="ExternalInput").ap()
    d["woT"] = nc.dram_tensor("woT", (128, MPC, D), BF16,
                              kind="ExternalInput").ap()
    # trig: [:,0]=cos*nwq, [:,1]=cos*nwk, [:,2]=swapped-sin*nwq, [:,3]=..nwk
    d["trig"] = nc.dram_tensor("trig", (64, 4, T), BF16,
                               kind="ExternalInput").ap()
    # per-head norm weights for pass rows: [:,0]=q, [:,1]=k (rows 0:64 == 1)
    d["snw"] = nc.dram_tensor("snw", (128, 2), F32,
                              kind="ExternalInput").ap()
    # conv weights packed [128, proj(3), m(2), tap(4)]
    d["convw"] = nc.dram_tensor("convw", (128, 3, MPC, KCONV), F32,
                                kind="ExternalInput").ap()
    # causal mask strip: mask[kl, j] = 1.0 iff kl <= j - 384
    d["mask4"] = nc.dram_tensor("mask4", (128, 896), BF16,
                                kind="ExternalInput").ap()
    outT = nc.dram_tensor("outT", (D, T), BF16,
                          kind="ExternalOutput").ap()
    dbg = {}
    if _DEBUG:
        dbg["dbg_qf"] = nc.dram_tensor(
            "dbg_qf", (128, MPC, T), BF16, kind="ExternalOutput").ap()
        dbg["dbg_kf"] = nc.dram_tensor(
            "dbg_kf", (128, MPC, T), BF16, kind="ExternalOutput").ap()
        dbg["dbg_vtr"] = nc.dram_tensor(
            "dbg_vtr", (128, MPC, NQ, 4, 128), BF16,
            kind="ExternalOutput").ap()
        dbg["dbg_svq"] = nc.dram_tensor(
            "dbg_svq", (128, MPC, T), BF16, kind="ExternalOutput").ap()
        dbg["dbg_rawq"] = nc.dram_tensor(
            "dbg_rawq", (128, MPC, T + PAD), BF16,
            kind="ExternalOutput").ap()

    inv_sqrt_hd = 1.0 / np.sqrt(HD)

    with ExitStack() as stk:
        tc = stk.enter_context(tile.TileContext(nc))
        if True:
            consts = stk.enter_context(tc.tile_pool(name="consts", bufs=1))
            rawp = stk.enter_context(tc.tile_pool(name="raw", bufs=1))
            svp = stk.enter_context(tc.tile_pool(name="sv", bufs=1))
            finp = stk.enter_context(tc.tile_pool(name="fin", bufs=1))
            wop = stk.enter_context(tc.tile_pool(name="wo", bufs=1))
            psacc = stk.enter_context(
                tc.tile_pool(name="psacc", bufs=4, space="PSUM"))
            pssum = stk.enter_context(
                tc.tile_pool(name="pssum", bufs=1, space="PSUM"))
            pssm = stk.enter_context(
                tc.tile_pool(name="pssm", bufs=3, space="PSUM"))
            # ---- constants ----
            trig_t = consts.tile([64, 4, T], BF16)
            nc.sync.dma_start(trig_t, d["trig"])
            mask4_t = consts.tile([128, 896], BF16)
            nc.scalar.dma_start(mask4_t, d["mask4"])
            convw_t = consts.tile([128, 3, MPC, KCONV], F32)
            nc.sync.dma_start(convw_t, d["convw"])
            snw_t = consts.tile([128, 2], F32)
            nc.scalar.dma_start(snw_t, d["snw"])
            ones_hd = consts.tile([128, 1], BF16)
            nc.vector.memset(ones_hd, 1.0)
            woT_t = wop.tile([128, MPC, D], BF16)
            nc.sync.dma_start(woT_t, d["woT"])

            # ---- persistent buffers ----
            rawq = rawp.tile([128, MPC, T + PAD], BF16)
            rawk = rawp.tile([128, MPC, T + PAD], BF16)
            rawv = rawp.tile([128, MPC, T + PAD], BF16)
            for r in (rawq, rawk, rawv):
                nc.vector.memset(r[:, :, 0:PAD], 0.0)
            raws = (rawq, rawk, rawv)
            # silu outputs (q/k get roped in place; v feeds the transpose)
            svq = svp.tile([128, MPC, T], BF16)
            svk = svp.tile([128, MPC, T], BF16)
            vv = svp.tile([128, MPC, T], BF16)
            svs = (svq, svk, vv)
            # final q/k in head-transposed layout [HD, m, T]
            qfT = finp.tile([128, MPC, T], BF16)
            kfT = finp.tile([128, MPC, T], BF16)
            fins = (qfT, kfT)
            # v^T per 512-block, stride-4 interleave: t = 512*b + 4*p + c
            vtr = finp.tile([128, MPC, NQ, 4, 128], BF16)

            groups = [(0, 0), (0, 1), (1, 0), (1, 1), (2, 0), (2, 1)]

            wqkvp = stk.enter_context(tc.tile_pool(name="wqkv", bufs=1))
            xp = stk.enter_context(tc.tile_pool(name="xb", bufs=2))
            convp = stk.enter_context(tc.tile_pool(name="conv", bufs=3))
            sqp = stk.enter_context(tc.tile_pool(name="sq", bufs=4))
            spp = stk.enter_context(tc.tile_pool(name="sp", bufs=8))
            rrp = stk.enter_context(tc.tile_pool(name="rrb", bufs=2))
            rbcp = stk.enter_context(tc.tile_pool(name="rbc", bufs=8))
            expp = stk.enter_context(tc.tile_pool(name="exp", bufs=3))
            attnp = stk.enter_context(tc.tile_pool(name="attn", bufs=2))
            ostp = stk.enter_context(tc.tile_pool(name="ostage", bufs=2))
            smp = stk.enter_context(tc.tile_pool(name="small", bufs=2))
            if True:
                w_all = wqkvp.tile([128, KD, 3, CPC], BF16)

                def phaseA_loads(tq, first=False):
                    xb = xp.tile([128, KD, NT], BF16, name="xb", tag="xb")
                    for k in range(KD):
                        if first:
                            for pi, wd in enumerate((d["wqT"], d["wkT"],
                                                     d["wvT"])):
                                deng = nc.sync if (k * 3 + pi) % 2 == 0 \
                                    else nc.scalar
                                deng.dma_start(
                                    w_all[:, k, pi, :],
                                    wd[k * 128:(k + 1) * 128, :])
                        deng = (nc.sync if k % 2 == 0 else nc.scalar) \
                            if tq < 2 else nc.sync
                        deng.dma_start(
                            xb[:, k, :],
                            d["xT"][k * 128:(k + 1) * 128,
                                    tq * NT:(tq + 1) * NT])
                    return xb

                def phaseA_mms(tq, xb, drain_eng):
                    # 6 simultaneous accumulations (3 psacc + 3 pssm banks)
                    pst = [psacc.tile([128, NT], F32, tag="acc",
                                      name=f"accA{gi}") for gi in range(3)] \
                        + [pssm.tile([128, NT], F32, tag="sm",
                                     name=f"accB{gi}") for gi in range(3)]
                    for k in range(KD):
                        for gi, (pi, m) in enumerate(groups):
                            nc.tensor.matmul(
                                pst[gi],
                                w_all[:, k, pi, m * 128:(m + 1) * 128],
                                xb[:, k, :],
                                start=(k == 0), stop=(k == KD - 1))
                    for gi, (pi, m) in enumerate(groups):
                        dst = raws[pi][:, m,
                                       PAD + tq * NT:PAD + (tq + 1) * NT]
                        nc.vector.tensor_copy(dst, pst[gi])

                def conv_silu_sq(pi, m, h, sqtiles):
                    """conv + silu (+square for q/k) on half h."""
                    base = h * HT
                    raw = raws[pi]
                    t0 = convp.tile([128, HT], BF16, tag="cvA", name="cv0")
                    nc.vector.tensor_scalar_mul(
                        t0, raw[:, m, base:base + HT],
                        convw_t[:, pi, m, 0:1])
                    for j in (1, 2, 3):
                        t1 = convp.tile([128, HT], BF16,
                                        tag=("cvB", "cvA")[j % 2], name="cvj")
                        nc.vector.scalar_tensor_tensor(
                            t1, raw[:, m, base + j:base + j + HT],
                            convw_t[:, pi, m, j:j + 1], t0,
                            mybir.AluOpType.mult, mybir.AluOpType.add)
                        t0 = t1
                    sv = svs[pi]
                    nc.scalar.activation(
                        sv[:, m, base:base + HT], t0,
                        mybir.ActivationFunctionType.Silu)
                    if pi < 2:
                        sq = sqp.tile([128, HT], BF16, tag="sq")
                        nc.scalar.activation(
                            sq, sv[:, m, base:base + HT],
                            mybir.ActivationFunctionType.Square,
                            scale=inv_sqrt_hd)
                        sqtiles[(pi, m, h)] = sq

                def phaseBh(h, sqtiles):
                    for m in range(MPC):
                        for pi in range(3):
                            conv_silu_sq(pi, m, h, sqtiles)

                def phaseBs_pair(s0, sqtiles):
                    """Finalize slices s0, s0+1: rstd, rope -> qfT/kfT.

                    Staged so the scalar queue sees Ln x8 then Exp x8 (one
                    table load each), and GpSimd's rope products run while
                    the scalar engine computes rstd.
                    """
                    combos = [(s, m, pi) for s in (s0, s0 + 1)
                              for m in range(MPC) for pi in range(2)]
                    ps_ss, sps, rrbs, rbcs = {}, {}, {}, {}
                    for cb in combos:
                        s, m, pi = cb
                        sl = slice(s * NT, (s + 1) * NT)
                        sv = svs[pi][:, m, sl]
                        sq = sqtiles[(pi, m, s // 2)]
                        ps = pssm.tile([1, NT], F32, tag="sm", name="ps_ss")
                        nc.tensor.matmul(
                            ps, ones_hd,
                            sq[:, (s % 2) * NT:(s % 2 + 1) * NT],
                            start=True, stop=True)
                        ps_ss[cb] = ps
                        # rope: swapped sin products on gpsimd
                        sp = spp.tile([64, NT], BF16, tag="sp", name="sp")
                        nc.gpsimd.tensor_mul(
                            sp[0:32, :], sv[32:64, :],
                            trig_t[32:64, 2 + pi, sl])
                        nc.gpsimd.tensor_mul(
                            sp[32:64, :], sv[0:32, :],
                            trig_t[0:32, 2 + pi, sl])
                        sps[cb] = sp
                        # cos product in place (after gpsimd reads sv)
                        nc.vector.tensor_mul(
                            sv[0:64, :], sv[0:64, :], trig_t[0:64, pi, sl])
                    for cb in combos:  # Ln batch (one table load)
                        nc.scalar.activation(
                            ps_ss[cb], ps_ss[cb],
                            mybir.ActivationFunctionType.Ln)
                    for cb in combos:  # Exp batch; rstd = exp(-0.5*ln(ms))
                        rrb = rrp.tile([1, NT], BF16, tag="rrb", name="rrb")
                        nc.scalar.activation(
                            rrb, ps_ss[cb], mybir.ActivationFunctionType.Exp,
                            scale=-0.5)
                        rbc = rbcp.tile([128, NT], BF16, tag="rbc",
                                        name="rbc")
                        nc.gpsimd.partition_broadcast(rbc, rrb)
                        rbcs[cb] = rbc
                    for cb in combos:  # add sin part, then fin
                        s, m, pi = cb
                        sl = slice(s * NT, (s + 1) * NT)
                        sv = svs[pi][:, m, sl]
                        nc.vector.tensor_add(sv[0:64, :], sv[0:64, :],
                                             sps[cb])
                        nc.vector.scalar_tensor_tensor(
                            fins[pi][:, m, sl], sv,
                            snw_t[:, pi:pi + 1], rbcs[cb],
                            mybir.AluOpType.mult, mybir.AluOpType.mult)

                def v_transpose(b):
                    for m in range(MPC):
                        nc.sync.dma_start_transpose(
                            vtr[:, m, b],
                            vv[:, m, b * NT:(b + 1) * NT])

                def phaseC(t, interleave=None):
                    qsl = slice(t * NT, (t + 1) * NT)
                    nch = 4 * (t + 1)
                    attn_m = []
                    for m in range(MPC):
                        ps_attn = psacc.tile([128, NT], F32, tag="acc",
                                             name="ps_attn")
                        ps_sum = pssum.tile([1, NT], F32, tag="sum1",
                                            name="ps_sum")

                        def qk(kc):
                            ps_s = pssm.tile([128, NT], F32, tag="sm",
                                             name="ps_s")
                            nc.tensor.matmul(
                                ps_s,
                                kfT[:, m, kc * 128:(kc + 1) * 128],
                                qfT[:, m, qsl], start=True, stop=True)
                            e = expp.tile([128, NT], BF16, tag="e", name="e")
                            nc.scalar.activation(
                                e, ps_s, mybir.ActivationFunctionType.Exp,
                                scale=inv_sqrt_hd)
                            dd = kc * 128 - t * NT
                            if dd >= 0:  # diagonal chunk: causal mask
                                nc.vector.tensor_mul(
                                    e, e, mask4_t[:, 384 - dd:896 - dd])
                            return e

                        epipe = [qk(kc) for kc in range(min(2, nch))]
                        for kc in range(nch):
                            if kc + 2 < nch:
                                epipe.append(qk(kc + 2))
                            e = epipe.pop(0)
                            b, c = kc // 4, kc % 4
                            nc.tensor.matmul(
                                ps_attn, vtr[:, m, b, c, :], e,
                                start=(kc == 0), stop=(kc == nch - 1))
                            nc.tensor.matmul(
                                ps_sum, ones_hd, e,
                                start=(kc == 0), stop=(kc == nch - 1))
                        # normalize by 1/sumexp via stride-0 DMA broadcast
                        rrf = smp.tile([1, NT], F32, tag="rrf", name="rrf")
                        nc.vector.reciprocal_approx_fast(rrf, ps_sum)
                        rrc = smp.tile([1, NT], BF16, tag="rrc", name="rrc")
                        nc.vector.tensor_copy(rrc, rrf)
                        rbc = rbcp.tile([128, NT], BF16, tag="rbc",
                                        name="rbcC")
                        nc.gpsimd.partition_broadcast(rbc, rrc)
                        am = attnp.tile([128, NT], BF16, tag="am", name="am")
                        nc.vector.tensor_mul(am, ps_attn, rbc)
                        attn_m.append(am)
                        if interleave:
                            interleave.pop(0)()
                    # output projection (wo resident)
                    for i in range(KD):
                        ps_o = psacc.tile([128, NT], F32, tag="acc",
                                          name="ps_o")
                        for j in range(MPC):
                            nc.tensor.matmul(
                                ps_o, woT_t[:, j, i * 128:(i + 1) * 128],
                                attn_m[j], start=(j == 0),
                                stop=(j == MPC - 1))
                        ost = ostp.tile([128, NT], BF16, tag="ost",
                                        name="ost")
                        if i % 4 == 3:
                            nc.scalar.activation(
                                ost, ps_o, mybir.ActivationFunctionType.Copy)
                        else:
                            nc.vector.tensor_copy(ost, ps_o)
                        deng = nc.sync if i % 2 == 0 else nc.gpsimd
                        deng.dma_start(outT[i * 128:(i + 1) * 128, qsl],
                                       ost)
                        if interleave:
                            interleave.pop(0)()

                # ================= emission schedule =================
                sqtiles = {}
                xb0 = phaseA_loads(0, first=True)
                xb1 = phaseA_loads(1)
                phaseA_mms(0, xb0, "v")
                phaseA_mms(1, xb1, "v")
                xb2 = phaseA_loads(2)
                xb3 = phaseA_loads(3)
                phaseBh(0, sqtiles)       # conv/silu/sq for t in [0, 1024)
                phaseA_mms(2, xb2, "g")
                phaseA_mms(3, xb3, "g")
                phaseBs_pair(0, sqtiles)
                v_transpose(0)
                v_transpose(1)

                # Bh1 pieces interleaved into C0/C1 emission
                pieces = []
                for m in range(MPC):
                    for pi in range(3):
                        pieces.append(
                            lambda pi=pi, m=m: conv_silu_sq(pi, m, 1,
                                                            sqtiles))
                nfill = 2 * MPC + 2 * KD  # interleave slots in C0+C1
                while len(pieces) < nfill:
                    pieces.append(lambda: None)
                phaseC(0, interleave=pieces)
                phaseC(1, interleave=pieces)
                assert not pieces
                phaseBs_pair(2, sqtiles)
                v_transpose(2)
                v_transpose(3)
                phaseC(2)
                phaseC(3)
                if _DEBUG:
                    nc.sync.dma_start(dbg["dbg_qf"], qfT)
                    nc.sync.dma_start(dbg["dbg_kf"], kfT)
                    nc.sync.dma_start(dbg["dbg_vtr"], vtr)
                    nc.sync.dma_start(dbg["dbg_svq"], svq)
                    nc.sync.dma_start(dbg["dbg_rawq"], rawq)

    nc.compile()
    return nc


def _prep_inputs(hidden_states, cos, sin, Wq, Wk, Wv, Wo,
                 conv_q_w, conv_k_w, conv_v_w, q_norm_w, k_norm_w):
    f = np.float32
    bf = ml_dtypes.bfloat16
    x = np.asarray(hidden_states, f)[0]            # [T, D]
    xT = np.ascontiguousarray(x.T.astype(bf))      # [D, T] bf16
    WqT = np.ascontiguousarray(np.asarray(Wq, f).T.astype(bf))
    WkT = np.ascontiguousarray(np.asarray(Wk, f).T.astype(bf))
    WvT = np.ascontiguousarray(np.asarray(Wv, f).T.astype(bf))
    WoT = np.asarray(Wo, f).T                      # [CPC(full D), D]

    cosT = np.asarray(cos, f)[0].T                 # [RD, T]
    sinT = np.asarray(sin, f)[0].T
    nwq = np.asarray(q_norm_w, f)
    nwk = np.asarray(k_norm_w, f)

    # trig tables with norm weights folded into the rotary rows.
    # sin table indexed by SOURCE row r (out row p = r xor 32):
    #   r in 0:32  -> p = r+32: +sin[p]*nw[p]
    #   r in 32:64 -> p = r-32: -sin[p]*nw[p]
    def mk_trig(nw):
        cosb = cosT * nw[0:RD, None]
        ss = np.zeros((RD, T), f)
        ss[0:32] = sinT[32:64] * nw[32:64, None]
        ss[32:64] = -sinT[0:32] * nw[0:32, None]
        return cosb, ss

    cosq, ssq = mk_trig(nwq)
    cosk, ssk = mk_trig(nwk)
    trig = np.stack([cosq, cosk, ssq, ssk], axis=1).astype(bf)  # [64,4,T]

    snw = np.ones((128, 2), f)
    snw[RD:128, 0] = nwq[RD:128]
    snw[RD:128, 1] = nwk[RD:128]

    # causal mask strip: mask[kl, j] = 1.0 iff kl <= j - 384
    pp = np.arange(128, dtype=f)[:, None]
    jj = np.arange(896, dtype=f)[None, :]
    mask4 = (pp <= jj - 384).astype(bf)

    in_maps = []
    for ci in range(NCORES):
        sl = slice(ci * CPC, (ci + 1) * CPC)
        convw = np.zeros((128, 3, MPC, KCONV), f)
        for pi, cw in enumerate((conv_q_w, conv_k_w, conv_v_w)):
            convw[:, pi] = np.asarray(cw, f)[sl].reshape(MPC, 128, KCONV
                                                         ).transpose(1, 0, 2)
        wo_res = np.ascontiguousarray(
            WoT[sl].reshape(MPC, 128, D).transpose(1, 0, 2).astype(bf))
        in_maps.append({
            "xT": xT,
            "wqT": np.ascontiguousarray(WqT[:, sl]),
            "wkT": np.ascontiguousarray(WkT[:, sl]),
            "wvT": np.ascontiguousarray(WvT[:, sl]),
            "woT": wo_res,
            "trig": trig,
            "snw": snw,
            "convw": np.ascontiguousarray(convw),
            "mask4": np.ascontiguousarray(mask4),
        })
    return in_maps


def kernel(hidden_states, cos, sin, Wq, Wk, Wv, Wo,
           conv_q_w, conv_k_w, conv_v_w, q_norm_w, k_norm_w,
           _trace=False):
    global _COMPILED
    if _COMPILED is None:
        _COMPILED = _build()
    nc = _COMPILED
    in_maps = _prep_inputs(hidden_states, cos, sin, Wq, Wk, Wv, Wo,
                           conv_q_w, conv_k_w, conv_v_w, q_norm_w, k_norm_w)
    res = bass_utils.run_bass_kernel_spmd(
        nc, in_maps, core_ids=list(range(NCORES)), trace=_trace)
    if _DEBUG:
        global _DEBUG_RESULTS
        _DEBUG_RESULTS = res.results
    acc = np.zeros((D, T), np.float64)
    for r in res.results:
        acc += np.asarray(r["outT"], np.float64)
    out = np.ascontiguousarray(acc.T.astype(np.float32))[None]
    if _trace:
        kernel._last_results = res
    return out
